# revision 15
# baseline (speedup 1.0000x reference)
"""GroupViT cross-attention layer on 8 TRN2 NeuronCores.

Data-parallel over batch (16 batches -> 2 per core, zero collectives).
Feature-major layout on chip: activations [feature(partition), token(free)],
weights host-transposed so every matmul contracts over the partition dim.

v4 (from 669us v2 baseline; v3 hit HAM-cold at 614us):
- scores: row-tiled pairs (K=64 per head at array rows 0/64) -- two heads'
  score matmuls run concurrently instead of zero-padding K to 128.
- ctx (probs @ V): fp8 DoubleRow over key-chunk pairs (96 MMs/batch).
- softmax exp split: even heads on ACT (native Exp -> fp8), odd heads on
  DVE via one-op Schraudolph exp: uint8(round(x*8*log2e + B)) IS the
  fp8e4m3 bit pattern of ~exp(x); DVE f32->uint8 rounds-to-nearest and
  saturates (underflow -> +0.0).  The denominator is computed from the
  same stored probs so the ~3% log-noise cancels in normalization
  (measured host-side: 2e-4 output contribution).
- HAM fix: the attention c-loop alone has PE duty <90% (exp-paced), which
  drops the PE clock to 1.2 GHz.  Batch 1's Q/V projections and batch 0's
  O-projection are emitted as filler INSIDE the c-loops so the PE queue
  always has independent dense matmul work.
- MLP in bf16 (fp8 MLP measured 2e-2 output error -- dominates the budget),
  weights streamed per fo-chunk as in v2.
- attention fp8 weights pre-scaled by powers of 2 out of the subnormal
  range (w ~ 0.02 std is below fp8e4m3 min normal 2^-6); scales undone
  for free via ACT `scale` and fused scalar multiplies.
- residual stream bf16; LN squares/outputs bf16 (2x DVE mode); denominator
  pipeline batched per chunk ([1,2,T] reciprocal).
"""

import numpy as np
import ml_dtypes

B, T, S, D, H, HD, FF = 16, 512, 2048, 768, 12, 64, 3072
NCORES = 8
BPC = B // NCORES
P = 128
DC = D // P            # 6 feature chunks
KP = DC // 2           # 3 doublerow k-pairs
SC = S // P            # 16 key-token chunks
SP = SC // 2           # 8 key-token chunk pairs
FFC = FF // P          # 24
EPS = 1e-5
SCALE = HD ** -0.5

# power-of-2 scales to lift fp8 weights out of the subnormal range
MQ, MK, MV, MO, MCTX = 64.0, 32.0, 32.0, 32.0, 32.0
S_EXP = 1.0 / (MQ * MK)          # undo q/k weight scaling inside exp
R_O = 1.0 / (MO * MCTX)          # undo o-proj + ctx scaling in residual add
# Schraudolph fp8 exp constants: bits = round(x*S_EXP*8*log2e + 55.656)
A_DVE = 8.0 / np.log(2.0) * S_EXP
B_DVE = 8.0 * 7 - 8.0 * 0.0430

_cached = {}


def _build(use_bias: bool):
    import concourse.bacc as bacc
    import concourse.tile as tile
    import concourse.mybir as mybir

    f32 = mybir.dt.float32
    bf16 = mybir.dt.bfloat16
    fp8 = mybir.dt.float8e4
    u8 = mybir.dt.uint8
    AF = mybir.ActivationFunctionType
    ALU = mybir.AluOpType
    DR = mybir.MatmulPerfMode.DoubleRow

    nc = bacc.Bacc("TRN2", target_bir_lowering=False, debug=False,
                   num_devices=NCORES)

    # ---- DRAM I/O (per-core shapes, host pre-tiled) ----
    qT8_d = nc.dram_tensor("qT8", [BPC, P, KP, 2, T], fp8, kind="ExternalInput")
    qrT_d = nc.dram_tensor("qrT", [BPC, P, DC, T], bf16, kind="ExternalInput")
    kT8_d = nc.dram_tensor("kT8", [BPC, P, KP, 2, S], fp8, kind="ExternalInput")
    w8q_d = nc.dram_tensor("w8q", [P, KP, 2, D], fp8, kind="ExternalInput")
    w8k_d = nc.dram_tensor("w8k", [P, KP, 2, D], fp8, kind="ExternalInput")
    w8v_d = nc.dram_tensor("w8v", [P, KP, 2, D], fp8, kind="ExternalInput")
    w8o_d = nc.dram_tensor("w8o", [P, KP, 2, D], fp8, kind="ExternalInput")
    fc1_d = nc.dram_tensor("fc1_t", [FFC, P, DC, P], bf16,
                           kind="ExternalInput")
    fc2_d = nc.dram_tensor("fc2_t", [FFC, P, D], bf16, kind="ExternalInput")
    # LN rows packed [4, D] bf16: 0=ln2g 1=-ln2b 2=lnpg 3=-lnpb
    lnrows_d = nc.dram_tensor("lnrows", [4, D], bf16, kind="ExternalInput")
    if use_bias:
        bq_d = nc.dram_tensor("bqv", [P, DC], f32, kind="ExternalInput")
        bk_d = nc.dram_tensor("bkv", [P, DC], f32, kind="ExternalInput")
        bo_d = nc.dram_tensor("bov", [P, DC], f32, kind="ExternalInput")
        bv_d = nc.dram_tensor("bvv", [1, D], f32, kind="ExternalInput")
        f1b_d = nc.dram_tensor("f1b", [P, FFC], f32, kind="ExternalInput")
        f2b_d = nc.dram_tensor("f2b", [P, DC], f32, kind="ExternalInput")
        onesr_d = nc.dram_tensor("ones_row", [1, P], f32, kind="ExternalInput")
    out_d = nc.dram_tensor("out", [BPC, P, DC, T], f32, kind="ExternalOutput")

    from contextlib import ExitStack

    with tile.TileContext(nc) as tc:
        with ExitStack() as stack:
            pool_specs = [
                ("small", 1), ("wres", 1), ("kinp", 2), ("qinp", 2),
                ("qrp", 2), ("qtp", 2), ("ktc", 3), ("vp", 2), ("expp", 3),
                ("ctxp", 2), ("xp", 2), ("hp", 2), ("x2p", 2), ("outp", 2),
                ("sqp", 1), ("mchp", 2), ("rdp", 1), ("bcp", 2),
                ("lnr", 2), ("tmp", 2), ("fstream", 2),
            ]
            pools = {nm: stack.enter_context(tc.tile_pool(name=nm, bufs=bu))
                     for nm, bu in pool_specs}
            (small, wres, kinp, qinp, qrp, qtp, ktc, vp, expp, ctxp, xp,
             hp, x2p, outp, sqp, mchp, rdp, bcp, lnr, tmpp, fstream) = (
                pools[nm] for nm, _ in pool_specs)

            # ---- persistent small tiles (all on-chip, no DMA) ----
            ones_col_bf = small.tile([P, 1], bf16, tag="ones_col_bf")
            nc.vector.memset(ones_col_bf[:], 1.0)
            ones_row_bf = small.tile([1, T], bf16, tag="ones_row_bf")
            nc.vector.memset(ones_row_bf[:], 1.0)
            ones64_bf = small.tile([1, HD], bf16, tag="ones64_bf")
            nc.vector.memset(ones64_bf[:], 1.0)
            eps_t = small.tile([1, 1], f32, tag="eps")
            nc.vector.memset(eps_t[:], EPS)

            ln2gb = small.tile([2, D], bf16, tag="ln2gb")
            nc.sync.dma_start(ln2gb[:], lnrows_d.ap()[0:2, :])
            lnpgb = small.tile([2, D], bf16, tag="lnpgb")
            nc.sync.dma_start(lnpgb[:], lnrows_d.ap()[2:4, :])

            if use_bias:
                bq_pc = small.tile([P, DC], f32, tag="bq_pc")
                nc.sync.dma_start(bq_pc[:], bq_d.ap())
                bk_pc = small.tile([P, DC], f32, tag="bk_pc")
                nc.sync.dma_start(bk_pc[:], bk_d.ap())
                bo_pc = small.tile([P, DC], f32, tag="bo_pc")
                nc.sync.dma_start(bo_pc[:], bo_d.ap())
                f1b_pc = small.tile([P, FFC], f32, tag="f1b_pc")
                nc.sync.dma_start(f1b_pc[:], f1b_d.ap())
                f2b_pc = small.tile([P, DC], f32, tag="f2b_pc")
                nc.sync.dma_start(f2b_pc[:], f2b_d.ap())
                bv_row = small.tile([1, D], f32, tag="bv_row")
                nc.sync.dma_start(bv_row[:], bv_d.ap())
                ones_r = small.tile([1, P], f32, tag="ones_r")
                nc.sync.dma_start(ones_r[:], onesr_d.ap())

            # ---- per-batch input DMAs issued first (see schedule) ----
            w8 = {}
            for nm, dram in (("q", w8q_d), ("k", w8k_d), ("v", w8v_d),
                             ("o", w8o_d)):
                t8 = wres.tile([P, KP, 2, D], fp8, tag=f"w8{nm}")
                nc.gpsimd.dma_start(t8[:], dram.ap())
                w8[nm] = t8

            # ---- per-batch persistent activations ----
            kin, qt, qr, v8, ctx8, xT, hT, x2T = {}, {}, {}, {}, {}, {}, {}, {}
            ktchs = {}

            # ============ attention building blocks (thunk lists) ============

            def attn_dma_thunk(b):
                def t():
                    kin[b] = kinp.tile([P, KP, 2, S], fp8, tag="kin",
                                       name=f"kin{b}")
                    nc.gpsimd.dma_start(kin[b][:], kT8_d.ap()[b])
                    qr[b] = qrp.tile([P, DC, T], bf16, tag="qr",
                                     name=f"qr{b}")
                    nc.sync.dma_start(qr[b][:], qrT_d.ap()[b])
                return [t]

            def qproj_thunks(b, ps1):
                qin8 = {}

                def dma():
                    qin8[0] = qinp.tile([P, KP, 2, T], fp8, tag="qin8",
                                        name=f"qin8_{b}")
                    nc.sync.dma_start(qin8[0][:], qT8_d.ap()[b])
                    qt[b] = qtp.tile([P, DC, T], bf16, tag="qt",
                                     name=f"qt{b}")

                def mk(mo):
                    def t():
                        ps = ps1.tile([P, T], f32, tag="ps1")
                        for kp in range(KP):
                            nc.tensor.matmul(
                                ps[:], w8["q"][:, kp, :, mo * P:(mo + 1) * P],
                                qin8[0][:, kp, :, :],
                                start=(kp == 0), stop=(kp == KP - 1),
                                perf_mode=DR)
                        dst = qt[b][:, mo, :]
                        if use_bias:
                            if mo % 2 == 0:
                                nc.scalar.activation(dst, ps[:], AF.Identity,
                                                     bias=bq_pc[:, mo:mo + 1])
                            else:
                                nc.vector.tensor_scalar_add(
                                    dst, ps[:], bq_pc[:, mo:mo + 1])
                        else:
                            nc.scalar.activation(dst, ps[:], AF.Copy)
                    return t
                return [dma] + [mk(mo) for mo in range(DC)]

            def vproj_thunks(b, ps1):
                """V projection + the v8 init memsets, one thunk per (so)."""
                def init():
                    v8[b] = vp.tile([P, SP, 2, H, HD + 4], fp8, tag="v8",
                                    name=f"v8_{b}")
                    vflat = v8[b][:].rearrange("p a b h e -> p (a b h) e")
                    nc.vector.memset(vflat[:, :, HD + 1:HD + 4], 0.0)
                    nc.vector.memset(vflat[:, :, HD:HD + 1], MV)
                    if use_bias:
                        bv_bc = small.tile([P, D], f32, tag="bv_bc",
                                           name=f"bvbc{b}")
                        v8[b + 10] = bv_bc
                        for half in range(2):
                            ps = ps1.tile([P, T], f32, tag="ps1")
                            nc.tensor.matmul(
                                ps[:, 0:384], ones_r[:],
                                bv_row[:, half * 384:(half + 1) * 384],
                                start=True, stop=True)
                            nc.vector.tensor_copy(
                                bv_bc[:, half * 384:(half + 1) * 384],
                                ps[:, 0:384])

                def mk(so, half):
                    def t():
                        ps = ps1.tile([P, T], f32, tag="ps1")
                        for kp in range(KP):
                            nc.tensor.matmul(
                                ps[:, 0:384],
                                kin[b][:, kp, :, so * P:(so + 1) * P],
                                w8["v"][:, kp, :,
                                        half * 384:(half + 1) * 384],
                                start=(kp == 0), stop=(kp == KP - 1),
                                perf_mode=DR)
                        dstv = v8[b][:, so // 2, so % 2,
                                     6 * half:6 * half + 6, 0:HD]
                        if use_bias:
                            nc.vector.tensor_tensor(
                                dstv, ps[:, 0:384],
                                v8[b + 10][:, half * 384:(half + 1) * 384],
                                ALU.add)
                        elif (so + half) % 2 == 0:
                            nc.scalar.activation(dstv, ps[:, 0:384],
                                                 AF.Copy)
                        else:
                            nc.vector.tensor_copy(dstv, ps[:, 0:384])
                    return t
                half0 = [mk(so, 0) for so in range(SC)]
                half1 = [mk(so, 1) for so in range(SC)]
                return [init] + half0, half1

            def kproj(b, c, ps1):
                ktch = ktc.tile([P, S], bf16, tag="ktc", name=f"k{b}_{c}")
                for st in range(4):
                    ps = ps1.tile([P, T], f32, tag="ps1")
                    for kp in range(KP):
                        nc.tensor.matmul(
                            ps[:], w8["k"][:, kp, :, c * P:(c + 1) * P],
                            kin[b][:, kp, :, st * T:(st + 1) * T],
                            start=(kp == 0), stop=(kp == KP - 1),
                            perf_mode=DR)
                    dst = ktch[:, st * T:(st + 1) * T]
                    if use_bias:
                        nc.scalar.activation(dst, ps[:], AF.Identity,
                                             bias=bk_pc[:, c:c + 1])
                    elif st % 2 == 0:
                        nc.scalar.activation(dst, ps[:], AF.Copy)
                    else:
                        nc.vector.tensor_copy(dst, ps[:])
                ktchs[(b, c)] = ktch

            def oproj_thunks(b, ps1):
                def init():
                    xT[b] = xp.tile([P, DC, T], bf16, tag="xT", name=f"xT{b}")

                def mk(mo):
                    def t():
                        ps = ps1.tile([P, T], f32, tag="ps1")
                        for kp in range(KP):
                            nc.tensor.matmul(
                                ps[:], w8["o"][:, kp, :, mo * P:(mo + 1) * P],
                                ctx8[b][:, kp, :, :],
                                start=(kp == 0), stop=(kp == KP - 1),
                                perf_mode=DR)
                        if use_bias:
                            tt = tmpp.tile([P, T], f32, tag="ot")
                            nc.vector.tensor_scalar(tt[:], ps[:], float(R_O),
                                                    bo_pc[:, mo:mo + 1],
                                                    ALU.mult, ALU.add)
                            nc.vector.tensor_tensor(xT[b][:, mo, :], tt[:],
                                                    qr[b][:, mo, :], ALU.add)
                        else:
                            nc.vector.scalar_tensor_tensor(
                                xT[b][:, mo, :], ps[:], float(R_O),
                                qr[b][:, mo, :], op0=ALU.mult, op1=ALU.add)
                    return t
                return [init] + [mk(mo) for mo in range(DC)]

            def run_filler(filler, n):
                for _ in range(n):
                    th = next(filler, None)
                    if th is not None:
                        th()

            def cloop(b, ps1, psSC, psCTX, filler, per_c):
                """Attention c-loop for batch b; pulls a burst of filler
                thunks at each c boundary to keep the PE queue dense (HAM
                warm) without blocking the sp pipeline on the ps1 ring."""
                ctx8[b] = ctxp.tile([P, KP, 2, T], fp8, tag="ctx8",
                                    name=f"ctx8_{b}")
                for c0 in range(2):
                    if (b, c0) not in ktchs:
                        kproj(b, c0, ps1)
                for c in range(DC):
                    ktch = ktchs.pop((b, c))
                    if c + 2 < DC:
                        kproj(b, c + 2, ps1)
                    pcx = [psCTX.tile([HD + 4, T], f32, tag="psCTX",
                                      name=f"pcx{i}") for i in range(2)]
                    for sp in range(SP):
                        pss = [psSC.tile([P, 2, T], f32, tag="psSC",
                                         name=f"pss{i}") for i in range(2)]
                        for j in range(2):
                            so = sp * 2 + j
                            nc.tensor.matmul(
                                pss[0][:, j, :],
                                ktch[0:HD, so * P:(so + 1) * P],
                                qt[b][0:HD, c, :], start=True, stop=True)
                            nc.tensor.matmul(
                                pss[1][:, j, :],
                                ktch[HD:P, so * P:(so + 1) * P],
                                qt[b][HD:P, c, :], start=True, stop=True)
                        exs = [expp.tile([P, 2, T], fp8, tag="exp",
                                         name=f"ex{i}") for i in range(2)]
                        # exp: even head on ACT, odd head on DVE (Schraudolph)
                        nc.scalar.activation(exs[0][:], pss[0][:], AF.Exp,
                                             scale=float(S_EXP))
                        nc.vector.tensor_scalar(
                            exs[1][:].bitcast(u8), pss[1][:],
                            float(A_DVE), float(B_DVE), ALU.mult, ALU.add)
                        for hh in range(2):
                            h = 2 * c + hh
                            nc.tensor.matmul(
                                pcx[hh][0:HD + 4, :],
                                v8[b][:, sp, :, h, :], exs[hh][:],
                                start=(sp == 0), stop=(sp == SP - 1),
                                perf_mode=DR)
                        if sp % 2 == 0:
                            run_filler(filler, 1)
                        # dummy weight loads: keep the PE activity monitor
                        # fed so the clock stays at 2.4 GHz even while the
                        # matmul rate is paced by the exp stream.  Each real
                        # matmul reloads its own weights, so these are
                        # harmless (~107ns each, no PSUM/DVE/ACT cost).
                        for _dl in range(5):
                            nc.tensor.ldweights(
                                w8["q"][:, _dl % KP, 0, 0:P])
                    for _dl in range(6):
                        nc.tensor.ldweights(w8["o"][:, _dl % KP, 0, 0:P])
                    # normalize both heads: den rows -> 1/den -> broadcast
                    den_t = rdp.tile([1, 2, T], f32, tag="den")
                    for hh in range(2):
                        nc.scalar.activation(den_t[0:1, hh, :],
                                             pcx[hh][HD:HD + 1, :], AF.Copy)
                    rden = rdp.tile([1, 2, T], f32, tag="rden")
                    nc.vector.reciprocal_approx_fast(out=rden[:], in_=den_t[:])
                    rden_bf = rdp.tile([1, 2, T], bf16, tag="rden_bf")
                    nc.vector.tensor_scalar_mul(rden_bf[:], rden[:],
                                                float(MCTX))
                    for hh in range(2):
                        psb = ps1.tile([P, T], f32, tag="ps1")
                        nc.tensor.matmul(psb[0:HD, :], ones64_bf[:],
                                         rden_bf[0:1, hh, :],
                                         start=True, stop=True)
                        bc_sb = bcp.tile([HD, T], bf16, tag="bc_sb")
                        nc.scalar.activation(bc_sb[:], psb[0:HD, :], AF.Copy)
                        nc.vector.tensor_tensor(
                            ctx8[b][hh * HD:(hh + 1) * HD, c // 2, c % 2, :],
                            pcx[hh][0:HD, :], bc_sb[:], ALU.mult)

            def ln_pass(jobs, ps_st, ps_bc):
                """LayerNorm over the feature(partition) dim for a list of
                (xsrc, gb_pair, dst_alloc) jobs, stage-interleaved so both
                batches' latency chains overlap on the engines.
                out = x*(g x rs) - (g*mu*rs - b) via two broadcast matmuls."""
                st = []
                for xsrc, gb_pair, dst_alloc in jobs:
                    psum_mu = ps_st.tile([1, T], f32, tag="st_mu")
                    psum_sq = ps_st.tile([1, T], f32, tag="st_sq")
                    for c2 in range(DC):
                        nc.tensor.matmul(psum_mu[:], ones_col_bf[:],
                                         xsrc[:, c2, :],
                                         start=(c2 == 0), stop=(c2 == DC - 1))
                    sqt = []
                    for c2 in range(DC):
                        sq = sqp.tile([P, T], bf16, tag="lnsq")
                        nc.vector.tensor_mul(sq[:], xsrc[:, c2, :],
                                             xsrc[:, c2, :])
                        sqt.append(sq)
                    for c2 in range(DC):
                        nc.tensor.matmul(psum_sq[:], ones_col_bf[:],
                                         sqt[c2][:],
                                         start=(c2 == 0), stop=(c2 == DC - 1))
                    st.append((psum_mu, psum_sq))
                rows = []
                for (xsrc, gb_pair, dst_alloc), (psum_mu, psum_sq) in zip(
                        jobs, st):
                    mu_t = lnr.tile([1, T], f32, tag="lnmu")
                    var_t = lnr.tile([1, T], f32, tag="lnvar")
                    rs_t = lnr.tile([1, T], bf16, tag="lnrs")
                    m1 = lnr.tile([2, T], bf16, tag="lnm1")
                    nc.vector.memset(m1[:], 1.0)
                    nc.scalar.activation(mu_t[:], psum_mu[:], AF.Identity,
                                         scale=1.0 / D)
                    nc.vector.tensor_mul(var_t[:], mu_t[:], mu_t[:])
                    nc.vector.scalar_tensor_tensor(
                        var_t[:], psum_sq[:], 1.0 / D, var_t[:],
                        op0=ALU.mult, op1=ALU.subtract)
                    nc.scalar.activation(rs_t[:], var_t[:],
                                         AF.Abs_reciprocal_sqrt,
                                         bias=eps_t[:])
                    nc.vector.tensor_tensor(m1[0:1, :], mu_t[:], rs_t[:],
                                            ALU.mult)
                    rows.append((rs_t, m1))
                for c2 in range(DC):
                    for (xsrc, gb_pair, dst_alloc), (rs_t, m1) in zip(
                            jobs, rows):
                        bcA = ps_bc.tile([P, T], f32, tag="ln_bcA")
                        bcB = ps_bc.tile([P, T], f32, tag="ln_bcB")
                        gsl = gb_pair[0:1, c2 * P:(c2 + 1) * P]
                        gbsl = gb_pair[:, c2 * P:(c2 + 1) * P]
                        nc.tensor.matmul(bcA[:], gsl, rs_t[:],
                                         start=True, stop=True)
                        nc.tensor.matmul(bcB[:], gbsl, m1[:],
                                         start=True, stop=True)
                        bcA_sb = bcp.tile([P, T], bf16, tag="bcA_sb")
                        bcB_sb = bcp.tile([P, T], bf16, tag="bcB_sb")
                        nc.scalar.activation(bcA_sb[:], bcA[:], AF.Copy)
                        nc.scalar.activation(bcB_sb[:], bcB[:], AF.Copy)
                        dst, finish = dst_alloc(c2)
                        tmp = tmpp.tile([P, T], bf16, tag="ln_tmp")
                        nc.vector.tensor_tensor(tmp[:], xsrc[:, c2, :],
                                                bcA_sb[:], ALU.mult)
                        nc.vector.tensor_tensor(dst, tmp[:], bcB_sb[:],
                                                ALU.subtract)
                        if finish is not None:
                            finish()

            def mlp(b, psF1, psF2):
                """bf16 MLP, weights streamed per fo-chunk."""
                x2T[b] = x2p.tile([P, DC, T], bf16, tag="x2T", name=f"x2T{b}")
                ps_f2 = [psF2.tile([P, T], f32, tag="psF2", name=f"ps_f2_{i}")
                         for i in range(DC)]
                for fo in range(FFC):
                    f1_sl = fstream.tile([P, DC, P], bf16, tag="f1_sl")
                    nc.sync.dma_start(f1_sl[:], fc1_d.ap()[fo])
                    f2_sl = fstream.tile([P, D], bf16, tag="f2_sl")
                    nc.sync.dma_start(f2_sl[:], fc2_d.ap()[fo])
                    ps1 = psF1.tile([P, T], f32, tag="psF1")
                    for ki in range(DC):
                        nc.tensor.matmul(ps1[:], f1_sl[:, ki, :],
                                         hT[b][:, ki, :],
                                         start=(ki == 0), stop=(ki == DC - 1))
                    mch = mchp.tile([P, T], bf16, tag="mch")
                    gbias = f1b_pc[:, fo:fo + 1] if use_bias else 0.0
                    nc.scalar.activation(mch[:], ps1[:], AF.Gelu, bias=gbias)
                    for mo in range(DC):
                        nc.tensor.matmul(
                            ps_f2[mo][:], f2_sl[:, mo * P:(mo + 1) * P],
                            mch[:],
                            start=(fo == 0), stop=(fo == FFC - 1))
                for mo in range(DC):
                    if use_bias:
                        tt = tmpp.tile([P, T], f32, tag="f2t")
                        nc.vector.tensor_scalar_add(tt[:], ps_f2[mo][:],
                                                    f2b_pc[:, mo:mo + 1])
                        nc.vector.tensor_tensor(x2T[b][:, mo, :], tt[:],
                                                xT[b][:, mo, :], ALU.add)
                    else:
                        nc.vector.tensor_tensor(x2T[b][:, mo, :], ps_f2[mo][:],
                                                xT[b][:, mo, :], ALU.add)

            # ================= schedule =================
            with (
                tc.tile_pool(name="ps1", bufs=2, space="PSUM") as ps1,
                tc.tile_pool(name="psSC", bufs=2, space="PSUM") as psSC,
                tc.tile_pool(name="psCTX", bufs=2, space="PSUM") as psCTX,
            ):
                # batch 0 head phase: dense PE
                v0a, v0b = vproj_thunks(0, ps1)
                for th in attn_dma_thunk(0) + qproj_thunks(0, ps1) + v0a + v0b:
                    th()
                # batch 0 c-loop: batch 1's dma/qproj/V-half0 as PE filler
                v1a, v1b = vproj_thunks(1, ps1)
                filler0 = iter(attn_dma_thunk(1) + qproj_thunks(1, ps1) + v1a)
                cloop(0, ps1, psSC, psCTX, filler0, 0)
                run_filler(filler0, 10**6)
                # batch 1 c-loop: V-half1 (heads 6-11, first used at c=3)
                # then batch 0's O-projection as filler
                filler1 = iter(v1b + oproj_thunks(0, ps1))
                cloop(1, ps1, psSC, psCTX, filler1, 0)
                run_filler(filler1, 10**6)
                for th in oproj_thunks(1, ps1):
                    th()

            with (
                tc.tile_pool(name="psST", bufs=2, space="PSUM") as psST,
                tc.tile_pool(name="psLB", bufs=2, space="PSUM") as psLB,
            ):
                jobs = []
                for b in range(BPC):
                    hT[b] = hp.tile([P, DC, T], bf16, tag="hT", name=f"hT{b}")
                    hview = hT[b][:]
                    jobs.append((xT[b], ln2gb[:],
                                 lambda c2, hv=hview: (hv[:, c2, :], None)))
                ln_pass(jobs, psST, psLB)

            with (
                tc.tile_pool(name="psF1", bufs=2, space="PSUM") as psF1,
                tc.tile_pool(name="psF2", bufs=6, space="PSUM") as psF2,
            ):
                for b in range(BPC):
                    mlp(b, psF1, psF2)

            with (
                tc.tile_pool(name="psST2", bufs=2, space="PSUM") as psST2,
                tc.tile_pool(name="psLB2", bufs=2, space="PSUM") as psLB2,
            ):
                jobs = []
                for b in range(BPC):
                    def out_alloc(c2, b=b):
                        t = outp.tile([P, T], f32, tag="outT")
                        fin = (lambda t=t, c2=c2, b=b:
                               nc.sync.dma_start(out_d.ap()[b][:, c2, :],
                                                 t[:]))
                        return t[:], fin
                    jobs.append((x2T[b], lnpgb[:], out_alloc))
                ln_pass(jobs, psST2, psLB2)

    nc.compile()
    return nc


def _get_nc(use_bias: bool):
    key = ("nc", use_bias)
    if key not in _cached:
        _cached[key] = _build(use_bias)
    return _cached[key]


def _to_fp8(x):
    return np.asarray(x, np.float32).astype(ml_dtypes.float8_e4m3)


def _to_bf16(x):
    return np.asarray(x, np.float32).astype(ml_dtypes.bfloat16)


def _tile_kp(wT):
    """[d_in, n] -> [P, KP, 2, n] with d_in = (kp*2 + i)*P + p."""
    n = wT.shape[1]
    return np.ascontiguousarray(wT.reshape(KP, 2, P, n).transpose(2, 0, 1, 3))


def _col_pc(v, nch):
    """[n] -> [P, nch] with n = c*P + p."""
    return np.ascontiguousarray(np.asarray(v, np.float32).reshape(nch, P).T)


def _prep_shared(wq, bq, wk, bk, wv, bv, wo, bo,
                 ln2_g, ln2_b, fc1_w, fc1_b, fc2_w, fc2_b, lnp_g, lnp_b,
                 use_bias):
    f = np.float32
    c = np.ascontiguousarray
    lnrows = np.stack([
        np.asarray(ln2_g, f), -np.asarray(ln2_b, f),
        np.asarray(lnp_g, f), -np.asarray(lnp_b, f)])
    shared = {
        "w8q": _to_fp8(_tile_kp(np.asarray(wq, f).T * f(SCALE * MQ))),
        "w8k": _to_fp8(_tile_kp(np.asarray(wk, f).T * f(MK))),
        "w8v": _to_fp8(_tile_kp(np.asarray(wv, f).T * f(MV))),
        "w8o": _to_fp8(_tile_kp(np.asarray(wo, f).T * f(MO))),
        "lnrows": _to_bf16(lnrows),
    }
    f1T = np.asarray(fc1_w, f).T           # [D, FF]
    f2T = np.asarray(fc2_w, f).T           # [FF, D]
    shared["fc1_t"] = _to_bf16(
        f1T.reshape(DC, P, FFC, P).transpose(2, 1, 0, 3))
    shared["fc2_t"] = _to_bf16(f2T.reshape(FFC, P, D))
    if use_bias:
        shared["bqv"] = _col_pc(np.asarray(bq, f) * f(SCALE * MQ), DC)
        shared["bkv"] = _col_pc(np.asarray(bk, f) * f(MK), DC)
        shared["bov"] = _col_pc(bo, DC)
        shared["bvv"] = c(np.asarray(bv, f).reshape(1, D) * f(MV))
        shared["f1b"] = _col_pc(fc1_b, FFC)
        shared["f2b"] = _col_pc(fc2_b, DC)
        shared["ones_row"] = np.ones((1, P), f)
    return shared


def _prep_batch(query_b, key_b):
    """Per-batch tensors: query_b [T, D], key_b [S, D]."""
    f = np.float32
    qT = np.asarray(query_b, f).T          # [D, T]
    kT = np.asarray(key_b, f).T            # [D, S]
    return (
        _to_fp8(qT.reshape(KP, 2, P, T).transpose(2, 0, 1, 3)),
        _to_bf16(qT.reshape(DC, P, T).transpose(1, 0, 2)),
        _to_fp8(kT.reshape(KP, 2, P, S).transpose(2, 0, 1, 3)),
    )


def kernel(query, key, wq, bq, wk, bk, wv, bv, wo, bo,
           ln2_g, ln2_b, fc1_w, fc1_b, fc2_w, fc2_b, lnp_g, lnp_b):
    from concourse.bass_utils import run_bass_kernel_spmd

    query = np.asarray(query, np.float32)
    key = np.asarray(key, np.float32)
    use_bias = any(bool(np.any(np.asarray(v))) for v in
                   (bq, bk, bv, bo, fc1_b, fc2_b))
    nc = _get_nc(use_bias)

    shared = _prep_shared(wq, bq, wk, bk, wv, bv, wo, bo,
                          ln2_g, ln2_b, fc1_w, fc1_b, fc2_w, fc2_b,
                          lnp_g, lnp_b, use_bias)
    in_maps = []
    for core in range(NCORES):
        m = dict(shared)
        q8s, qrs, k8s = [], [], []
        for j in range(BPC):
            b = core * BPC + j
            q8, qrr, k8 = _prep_batch(query[b], key[b])
            q8s.append(q8)
            qrs.append(qrr)
            k8s.append(k8)
        m["qT8"] = np.stack(q8s)
        m["qrT"] = np.stack(qrs)
        m["kT8"] = np.stack(k8s)
        in_maps.append(m)

    res = run_bass_kernel_spmd(nc, in_maps, core_ids=list(range(NCORES)))
    kernel._last_result = res
    out = np.stack([r["out"] for r in res.results])   # [NC, BPC, P, DC, T]
    # [core, b, p, c, t] -> [B, T, c*P+p]
    out = out.reshape(B, P, DC, T).transpose(0, 3, 2, 1).reshape(B, T, D)
    return np.ascontiguousarray(out)
